# revision 23
# baseline (speedup 1.0000x reference)
"""Grouped BERT self-attention on 8 TRN2 NeuronCores.

Problem: G=4 groups, B=4 batch, L=512 seq, C=768 (12 heads x 64).
Sharding: the 16 (g, b) attention problems are embarrassingly parallel;
each core handles one group g = core//2 and two batches. Weights are
per-group so each core loads exactly one group's weights. No collectives.

Per-(g,b) on-chip dataflow:
  q/k projections run in fp8(e4m3) DoubleRow: weights are host-scaled by
  64 (fp8 dynamic range) and packed [p, j, pass*2+i, d] so each DR matmul
  contracts 256 of C=768 per pass (3 passes/chunk, 2x bf16 throughput).
  The k bias is dropped entirely: the resulting per-query multiplicative
  factor exp(q.bk) cancels in the softmax normalization. The q bias is
  pre-scaled by 64 host-side so the PSUM drain stays a single
  tensor_scalar_add; the 64*64 scaling is folded into the exp scale
  (0.125/4096).
    qT[d,l] = (64 Wq[c,d]).T @ hst8[c,l] + 64 bq   (fp8 DR, bf16 out)
    kT[d,l] = (64 Wk[c,d]).T @ hst8[c,l]           (fp8 DR, bf16 out)
  v[m,d] = hstb[c,m].T @ Wv[c,d] + bv  stays bf16 (v feeds the output
    average directly; fp8 there would blow the 2e-2 error budget), stored
    [m, head, 65] with a ones column per head -> softmax denominator.
  ST[m,l] = kT[d,m].T @ qT[d,l]  (heads paired on partitions 0:64/64:128
    -> concurrent PE row-tiles, shared 2-bank PSUM tile)
  E[m,l]  = exp(ST/32768 + mask[m])  (one ACT op per head-pair, bf16 out)
  ctx[l, 2, d+1] = E[m,l].T @ v_aug[m, d+1]  (ctx-direct; column d=64
    catches the softmax denominator)
  out[l,:] = ctx * recip(denom)

PE emission interleaves score-pair matmuls of unit N with the PV matmuls
of unit N-2 so the in-order PE queue never waits on the ScalarEngine's
exp, and spreads the bf16 v-projection matmuls across units as PE filler
(the fp8 q/k path alone would under-fill the PE against the exp cadence
and let the HAM clock gate re-throttle mid-kernel). Input DMAs are
split/staged so tensors gating the first matmul group transfer first,
and dummy warm-up matmuls hold the PE activity monitor at full clock
until real work arrives.
"""

import numpy as np
import ml_dtypes

import concourse.bacc as bacc
import concourse.bass as bass
import concourse.tile as tile
import concourse.mybir as mybir
from concourse import bass_utils

# avoid FishPath artifact upload in the axon trace path
bass_utils.upload_artifacts = lambda tmpdir: tmpdir

G, B, L, C = 4, 4, 512, 768
NH, DH = 12, 64
NB = 2          # batches per core
CCH = C // 128  # 6 contraction chunks
NPASS = 3       # fp8 DoubleRow passes (256 contraction each)
LCH = L // 128  # 4 seq chunks
N_CORES = 8
WSCALE = 64.0   # fp8 weight pre-scale
EXP_SCALE = 0.125 / (WSCALE * WSCALE)

BF16 = mybir.dt.bfloat16
F8 = mybir.dt.float8e4
F32 = mybir.dt.float32
NPBF16 = ml_dtypes.bfloat16
NPF8 = ml_dtypes.float8_e4m3

_COMPILED = None


def _build():
    nc = bacc.Bacc("TRN2", target_bir_lowering=False, debug=False)
    AF = mybir.ActivationFunctionType
    DR = mybir.MatmulPerfMode.DoubleRow

    hst8_d = nc.declare_dram_parameter("hst8", [NB, 128, 2 * NPASS, L], F8, isOutput=False)
    hstb_d = nc.declare_dram_parameter("hstb", [NB, 128, CCH, L], BF16, isOutput=False)
    wq8_d = nc.declare_dram_parameter("wq8", [128, CCH, 2 * NPASS, 128], F8, isOutput=False)
    wk8_d = nc.declare_dram_parameter("wk8", [128, CCH, 2 * NPASS, 128], F8, isOutput=False)
    wv_d = nc.declare_dram_parameter("wv", [2, 128, CCH, C // 2], BF16, isOutput=False)
    bq64_d = nc.declare_dram_parameter("bq64", [128, CCH], F32, isOutput=False)
    bvb_d = nc.declare_dram_parameter("bvb", [128, C], BF16, isOutput=False)
    mask_d = nc.declare_dram_parameter("mask", [NB, 128, LCH], F32, isOutput=False)
    out_d = nc.declare_dram_parameter("out", [NB, LCH, 128, C], BF16, isOutput=True)

    with tile.TileContext(nc) as tc:
        with (
            tc.tile_pool(name="wpool", bufs=1) as wpool,
            tc.tile_pool(name="hpool", bufs=2) as hpool,
            tc.tile_pool(name="qkpool", bufs=2) as qkpool,
            tc.tile_pool(name="vpool", bufs=2 * LCH) as vpool,
            tc.tile_pool(name="epool", bufs=12) as epool,
            tc.tile_pool(name="cpool", bufs=2 * LCH) as cpool,
            tc.tile_pool(name="rpool", bufs=8) as rpool,
            tc.tile_pool(name="pqk", bufs=2, space=bass.MemorySpace.PSUM) as pqk,
            tc.tile_pool(name="pss", bufs=2, space=bass.MemorySpace.PSUM) as pss_pool,
            tc.tile_pool(name="ppv", bufs=2, space=bass.MemorySpace.PSUM) as ppv,
        ):
            # ---- persistent constants ----
            wq8 = wpool.tile([128, CCH, 2 * NPASS, 128], F8, tag="wq8")
            wk8 = wpool.tile([128, CCH, 2 * NPASS, 128], F8, tag="wk8")
            # wv split by output half so each half is one contiguous DMA
            # (a column-sliced DMA of one [128,CCH,C] tensor transfers in
            # 768B strided chunks and runs ~2x slower)
            wv = [wpool.tile([128, CCH, C // 2], BF16, tag=f"wv{h}", name=f"wv{h}")
                  for h in range(2)]
            bq64 = wpool.tile([128, CCH], F32, tag="bq64")
            bvb = wpool.tile([128, C], BF16, tag="bvb")
            hst8, hstb, msk, qt, kt, vt, e_of = {}, {}, {}, {}, {}, {}, {}
            for b in range(NB):
                hst8[b] = hpool.tile([128, 2 * NPASS, L], F8, tag="hst8", name=f"hst8_{b}")
                hstb[b] = hpool.tile([128, CCH, L], BF16, tag="hstb", name=f"hstb{b}")
                msk[b] = hpool.tile([128, LCH], F32, tag="mask", name=f"msk{b}")

            # staged input DMAs. Transfers on one queue run serially FIFO at
            # ~130GB/s; the three dynamic queues (sync/scalar/gpsimd) run
            # concurrently. Order per queue is a bandwidth schedule matched
            # to when each tensor gates PE work (first qk chunk ~9.5us,
            # v-projection pieces from ~15us, batch-1 tensors ~25us+).
            # sync: qk weights j0/j1 (contiguous j-pair slabs), rest of wq,
            # then wv halves
            nc.sync.dma_start(wq8[:, 0:2], wq8_d[:, 0:2])
            nc.sync.dma_start(wk8[:, 0:2], wk8_d[:, 0:2])
            nc.sync.dma_start(wq8[:, 2:6], wq8_d[:, 2:6])
            nc.sync.dma_start(wv[0][:], wv_d[0])
            nc.sync.dma_start(wv[1][:], wv_d[1])
            # scalar: hst8[0] as ONE contiguous transfer (pass-sliced DMAs
            # have 1KB strided runs and halve the queue's bandwidth), then
            # rest of wk, then b0 bf16 activations
            nc.scalar.dma_start(hst8[0][:], hst8_d[0])
            nc.scalar.dma_start(wk8[:, 2:6], wk8_d[:, 2:6])
            nc.scalar.dma_start(hstb[0][:, 0:3], hstb_d[0, :, 0:3])
            # gpsimd: small tensors, the other hstb[0] half, then batch 1
            nc.gpsimd.dma_start(msk[0][:], mask_d[0])
            nc.gpsimd.dma_start(bq64[:], bq64_d[:])
            nc.gpsimd.dma_start(hstb[0][:, 3:6], hstb_d[0, :, 3:6])
            nc.gpsimd.dma_start(bvb[:], bvb_d[:])
            nc.gpsimd.dma_start(msk[1][:], mask_d[1])
            nc.gpsimd.dma_start(hstb[1][:, 0:3], hstb_d[1, :, 0:3])
            nc.gpsimd.dma_start(hstb[1][:, 3:6], hstb_d[1, :, 3:6])
            nc.gpsimd.dma_start(hst8[1][:], hst8_d[1])

            def emit_v_piece(b, t, half):
                # one PSUM-group of the bf16 v projection: l-chunk t,
                # output half (6 of 12 heads). ~1us of PE filler.
                if half == 0 and t == 0:
                    vt[b] = [
                        vpool.tile([128, NH, DH + 1], BF16, tag="v", name=f"v{b}_{tt}")
                        for tt in range(LCH)
                    ]
                ncol = C // 2  # 384
                ps = pqk.tile([128, ncol], F32, tag="big", name="psv")
                for k in range(CCH):
                    nc.tensor.matmul(
                        ps[:],
                        hstb[b][:, k, 128 * t : 128 * (t + 1)],
                        wv[half][:, k, :],
                        start=(k == 0),
                        stop=(k == CCH - 1),
                    )
                nh2 = NH // 2
                nc.vector.tensor_add(
                    vt[b][t][:, half * nh2 : (half + 1) * nh2, 0:DH],
                    ps[:].rearrange("p (h d) -> p h d", d=DH),
                    bvb[:, half * ncol : (half + 1) * ncol].rearrange(
                        "p (h d) -> p h d", d=DH
                    ),
                )
                if half == 0:
                    nc.vector.memset(vt[b][t][:, :, DH : DH + 1], 1.0)

            def emit_qk_chunk(b, j):
                if j == 0:
                    qt[b] = qkpool.tile([128, CCH, L], BF16, tag="qt", name=f"qt{b}")
                    kt[b] = qkpool.tile([128, CCH, L], BF16, tag="kt", name=f"kt{b}")
                for w8, dst, bias in ((wq8, qt[b], bq64), (wk8, kt[b], None)):
                    ps = pqk.tile([128, L], F32, tag="big", name="psqk")
                    for p in range(NPASS):
                        nc.tensor.matmul(
                            ps[:],
                            w8[:, j, 2 * p : 2 * p + 2, :],
                            hst8[b][:, 2 * p : 2 * p + 2, :],
                            start=(p == 0),
                            stop=(p == NPASS - 1),
                            perf_mode=DR,
                        )
                    if bias is not None:
                        nc.vector.tensor_scalar_add(dst[:, j, :], ps[:], bias[:, j : j + 1])
                    else:
                        nc.vector.tensor_scalar_add(dst[:, j, :], ps[:], 0.0)

            def emit_scores_mc(b, hp, mc):
                if mc == 0:
                    e_of[(b, hp)] = [
                        epool.tile([128, 2, L], BF16, tag="e", name=f"e{b}_{hp}_{m}")
                        for m in range(LCH)
                    ]
                e = e_of[(b, hp)]
                ps = pss_pool.tile([128, 2, L], F32, tag="pss", name="pss")
                for h2 in range(2):
                    pr = slice(64 * h2, 64 * (h2 + 1))
                    nc.tensor.matmul(
                        ps[:, h2, :],
                        kt[b][pr, hp, 128 * mc : 128 * (mc + 1)],
                        qt[b][pr, hp, :],
                    )
                nc.scalar.activation(
                    e[mc][:].rearrange("p a l -> p (a l)"),
                    ps[:].rearrange("p a l -> p (a l)"), AF.Exp,
                    bias=msk[b][:, mc : mc + 1], scale=EXP_SCALE,
                )

            def emit_pv_pair(b, hp, lcs):
                # ctx-direct PV for a head pair over an l-chunk pair: one
                # PSUM bank holds both chunks' both heads' [128, 65]
                # results; denominators at column 64 of each -> one
                # [128,2,2] reciprocal + one broadcast-multiply into the
                # paired ctx tile.
                e = e_of[(b, hp)]
                pc = ppv.tile([128, 2, 2, DH + 1], F32, tag="pv", name="pc")
                for li, lc in enumerate(lcs):
                    for h2 in range(2):
                        head = 2 * hp + h2
                        for mc in range(LCH):
                            nc.tensor.matmul(
                                pc[:, li, h2, :],
                                e[mc][:, h2, 128 * lc : 128 * (lc + 1)],
                                vt[b][mc][:, head, :],
                                start=(mc == 0),
                                stop=(mc == LCH - 1),
                            )
                rec = rpool.tile([128, 2, 2, 1], F32, tag="rec", name="rec")
                nc.vector.reciprocal(rec[:], pc[:, :, :, DH : DH + 1])
                nc.vector.tensor_mul(
                    ctxs[(b, lcs[0] // 2)][:, :, 2 * hp : 2 * hp + 2, :],
                    pc[:, :, :, 0:DH],
                    rec[:].broadcast_to((128, 2, 2, DH)),
                )

            # ---- HAM warm-up: dummy matmuls on garbage SBUF keep the PE
            # active (half-clock) from program start so the activity monitor
            # releases the clock gate before the first real matmul ----
            warm = wpool.tile([128, 256], BF16, tag="warm")
            nc.vector.memset(warm[:], 0.0)
            # preload the Exp activation table off the critical path
            wexp = wpool.tile([128, 1], BF16, tag="wexp")
            nc.scalar.activation(wexp[:], warm[:, 0:1], AF.Exp, bias=0.0, scale=1.0)
            for i in range(16):
                pw = ppv.tile([1, 256], F32, tag="pv", name=f"pw{i}")
                nc.tensor.matmul(pw[:], warm[:, 0:1], warm[:])

            # ---- emission schedule ----
            units = []
            for b in range(NB):
                for hp in range(CCH):
                    units.append((b, hp))

            def emit_out(b, lcs):
                # spread the 0.2MB output DMAs across queues so no single
                # SWDGE issue chain (~0.7us each) sits on the critical path
                engs = {0: [nc.sync, nc.gpsimd, nc.sync, nc.gpsimd],
                        1: [nc.sync, nc.gpsimd, nc.scalar, nc.sync]}[b]
                for lc in lcs:
                    flat = ctxs[(b, lc // 2)][:, lc % 2].rearrange("p h d -> p (h d)")
                    engs[lc].dma_start(out_d[b, lc], flat[:])

            # ctx tiles paired over l-chunks (lc 0-1 and lc 2-3) so the PV
            # epilogue is one reciprocal + one multiply per head-pair
            ctxs = {}
            for b in range(NB):
                for lp in range(2):
                    ctxs[(b, lp)] = cpool.tile(
                        [128, 2, NH, DH], BF16, tag="ctx", name=f"ctx{b}_{lp}"
                    )
            from collections import deque
            # v-projection pieces double as PE filler between score groups:
            # unit 0 waits for hstb[0]/wv to land, so b0's pieces go in
            # units 0(end)-2, b1's spread across the middle units. qk for
            # unit N+1 is emitted inside unit N (between score groups) so
            # the ACT engine's exp stream never drains during a qk phase.
            # half-major order: PV for head-pair hp only reads v half hp//3,
            # so half-0 pieces unblock the first PV pairs while half-1
            # transfers are still in flight.
            vq = deque([(b, t, h) for b in range(NB) for h in range(2) for t in range(LCH)])
            # pieces popped after score-group 1 / group 2 of each unit;
            # unit 0 gets none (cold/DMA-gated), b0-half0 completes by
            # unit 2's first slot (PV reads it right after), b0-half1
            # waits for the wv half-1 transfer (~22us), rest flat.
            VFILL = {1: (0, 2), 2: (2, 0), 3: (2, 2), 4: (1, 1),
                     5: (1, 1), 6: (1, 1), 7: (1, 1), 8: (1, 1)}

            lag = deque()
            emit_qk_chunk(*units[0])
            for n, (b, hp) in enumerate(units):
                v1, v2 = VFILL.get(n, (0, 0))
                emit_scores_mc(b, hp, 0)
                emit_scores_mc(b, hp, 1)
                for _ in range(v1):
                    if vq:
                        emit_v_piece(*vq.popleft())
                if len(lag) >= 2:
                    emit_pv_pair(*lag[0], (0, 1))
                emit_scores_mc(b, hp, 2)
                if n + 1 < len(units):
                    emit_qk_chunk(*units[n + 1])
                emit_scores_mc(b, hp, 3)
                for _ in range(v2):
                    if vq:
                        emit_v_piece(*vq.popleft())
                if len(lag) >= 2:
                    pp = lag.popleft()
                    emit_pv_pair(*pp, (2, 3))
                    e_of.pop(pp)
                    if pp[1] == CCH - 1:
                        emit_out(pp[0], (0, 1, 2, 3))
                lag.append((b, hp))
            while lag:
                pp = lag.popleft()
                emit_pv_pair(*pp, (0, 1))
                if pp[1] == CCH - 1:
                    emit_out(pp[0], (0, 1))
                emit_pv_pair(*pp, (2, 3))
                e_of.pop(pp)
                if pp[1] == CCH - 1:
                    emit_out(pp[0], (2, 3))

    nc.compile()
    return nc


def _get_compiled():
    global _COMPILED
    if _COMPILED is None:
        _COMPILED = _build()
    return _COMPILED


def _prep_core(hs, mask, wq, wk, wv, bq, bv, g, b0):
    hs_gb = np.ascontiguousarray(hs[g, b0 : b0 + NB])  # [2, L, C]
    hs_cl = hs_gb.transpose(0, 2, 1)  # [2, C, L]
    # bf16 for the v projection: [b, p, k, l], c = 128k + p
    hstb = np.ascontiguousarray(
        hs_cl.reshape(NB, CCH, 128, L).transpose(0, 2, 1, 3)
    ).astype(NPBF16)
    # fp8 for q/k DoubleRow: [b, p, pass*2+i, l], c = 256*pass + 128*i + p
    hst8 = np.ascontiguousarray(
        hs_cl.reshape(NB, NPASS * 2, 128, L).transpose(0, 2, 1, 3)
    ).astype(NPF8)

    def wprep8(w):
        # [p, j, pass*2+i, d], c = 256*pass + 128*i + p, d = 128j + d'
        return np.ascontiguousarray(
            (w[g] * WSCALE).reshape(NPASS * 2, 128, CCH, 128).transpose(1, 2, 0, 3)
        ).astype(NPF8)

    def wprep(w):
        # output-half-major: [h, p, k, d'] = W[128k+p, 384h+d']
        return np.ascontiguousarray(
            w[g].reshape(CCH, 128, 2, C // 2).transpose(2, 1, 0, 3)
        ).astype(NPBF16)

    bq_t = np.ascontiguousarray(bq[g, 0].reshape(CCH, 128).T * WSCALE).astype(np.float32)
    bvb = np.ascontiguousarray(np.broadcast_to(bv[g, 0], (128, C))).astype(NPBF16)
    # mask[b, p, mc] = mask[g, b0+b, 0, 0, 128mc+p]
    msk = np.ascontiguousarray(
        mask[g, b0 : b0 + NB, 0, 0].reshape(NB, LCH, 128).transpose(0, 2, 1)
    ).astype(np.float32)
    return {
        "hst8": hst8,
        "hstb": hstb,
        "wq8": wprep8(wq),
        "wk8": wprep8(wk),
        "wv": wprep(wv),
        "bq64": bq_t,
        "bvb": bvb,
        "mask": msk,
    }


def kernel(
    hidden_states,
    attention_mask,
    query_weight,
    query_bias,
    key_weight,
    key_bias,
    value_weight,
    value_bias,
    _trace=False,
):
    hs = np.asarray(hidden_states, dtype=np.float32)
    mask = np.asarray(attention_mask, dtype=np.float32)
    wq = np.asarray(query_weight, dtype=np.float32)
    wk = np.asarray(key_weight, dtype=np.float32)
    wv = np.asarray(value_weight, dtype=np.float32)
    bq = np.asarray(query_bias, dtype=np.float32)
    bv = np.asarray(value_bias, dtype=np.float32)
    del key_bias  # exactly cancelled by the softmax normalization

    nc = _get_compiled()
    in_maps = []
    for c in range(N_CORES):
        g, b0 = c // 2, NB * (c % 2)
        in_maps.append(_prep_core(hs, mask, wq, wk, wv, bq, bv, g, b0))

    global _COMPILED
    res = None
    for attempt in range(3):
        try:
            res = bass_utils.run_bass_kernel_spmd(
                nc, in_maps, core_ids=list(range(N_CORES)), trace=_trace
            )
            # force materialization so device faults surface here
            for m in res.results:
                for v in m.values():
                    np.asarray(v)
            break
        except Exception:
            if attempt == 2:
                raise
            _COMPILED = None
            nc = _get_compiled()

    out = np.empty((G, B, L, C), dtype=np.float32)
    for c in range(N_CORES):
        g, b0 = c // 2, NB * (c % 2)
        o = res.results[c]["out"]  # [NB, LCH, 128, C] bf16
        out[g, b0 : b0 + NB] = o.reshape(NB, L, C).astype(np.float32)
    if _trace:
        kernel.last_exec_time_ns = res.exec_time_ns
    return out


# revision 25
# speedup vs baseline: 1.0175x; 1.0175x over previous
"""Grouped BERT self-attention on 8 TRN2 NeuronCores.

Problem: G=4 groups, B=4 batch, L=512 seq, C=768 (12 heads x 64).
Sharding: the 16 (g, b) attention problems are embarrassingly parallel;
each core handles one group g = core//2 and two batches. Weights are
per-group so each core loads exactly one group's weights. No collectives.

Per-(g,b) on-chip dataflow:
  q/k projections run in fp8(e4m3) DoubleRow: weights are host-scaled by
  64 (fp8 dynamic range) and packed [p, j, pass*2+i, d] so each DR matmul
  contracts 256 of C=768 per pass (3 passes/chunk, 2x bf16 throughput).
  The k bias is dropped entirely: the resulting per-query multiplicative
  factor exp(q.bk) cancels in the softmax normalization. The q bias is
  pre-scaled by 64 host-side so the PSUM drain stays a single
  tensor_scalar_add; the 64*64 scaling is folded into the exp scale
  (0.125/4096).
    qT[d,l] = (64 Wq[c,d]).T @ hst8[c,l] + 64 bq   (fp8 DR, bf16 out)
    kT[d,l] = (64 Wk[c,d]).T @ hst8[c,l]           (fp8 DR, bf16 out)
  v[m,d] = hstb[c,m].T @ Wv[c,d] + bv  stays bf16 (v feeds the output
    average directly; fp8 there would blow the 2e-2 error budget), stored
    [m, head, 65] with a ones column per head -> softmax denominator.
  ST[m,l] = kT[d,m].T @ qT[d,l]  (heads paired on partitions 0:64/64:128
    -> concurrent PE row-tiles, shared 2-bank PSUM tile)
  E[m,l]  = exp(ST/32768 + mask[m])  (one ACT op per head-pair, bf16 out)
  ctx[l, 2, d+1] = E[m,l].T @ v_aug[m, d+1]  (ctx-direct; column d=64
    catches the softmax denominator)
  out[l,:] = ctx * recip(denom)

PE emission interleaves score-pair matmuls of unit N with the PV matmuls
of unit N-2 so the in-order PE queue never waits on the ScalarEngine's
exp, and spreads the bf16 v-projection matmuls across units as PE filler
(the fp8 q/k path alone would under-fill the PE against the exp cadence
and let the HAM clock gate re-throttle mid-kernel). Input DMAs are
split/staged so tensors gating the first matmul group transfer first,
and dummy warm-up matmuls hold the PE activity monitor at full clock
until real work arrives.
"""

import numpy as np
import ml_dtypes

import concourse.bacc as bacc
import concourse.bass as bass
import concourse.tile as tile
import concourse.mybir as mybir
from concourse import bass_utils

# avoid FishPath artifact upload in the axon trace path
bass_utils.upload_artifacts = lambda tmpdir: tmpdir

G, B, L, C = 4, 4, 512, 768
NH, DH = 12, 64
NB = 2          # batches per core
CCH = C // 128  # 6 contraction chunks
NPASS = 3       # fp8 DoubleRow passes (256 contraction each)
LCH = L // 128  # 4 seq chunks
N_CORES = 8
WSCALE = 64.0   # fp8 weight pre-scale
EXP_SCALE = 0.125 / (WSCALE * WSCALE)

BF16 = mybir.dt.bfloat16
F8 = mybir.dt.float8e4
F32 = mybir.dt.float32
NPBF16 = ml_dtypes.bfloat16
NPF8 = ml_dtypes.float8_e4m3

_COMPILED = None


def _build():
    nc = bacc.Bacc("TRN2", target_bir_lowering=False, debug=False)
    AF = mybir.ActivationFunctionType
    DR = mybir.MatmulPerfMode.DoubleRow

    hst8_d = nc.declare_dram_parameter("hst8", [NB, 128, 2 * NPASS, L], F8, isOutput=False)
    hstb_d = nc.declare_dram_parameter("hstb", [NB, 128, CCH, L], BF16, isOutput=False)
    wq8_d = nc.declare_dram_parameter("wq8", [128, CCH, 2 * NPASS, 128], F8, isOutput=False)
    wk8_d = nc.declare_dram_parameter("wk8", [128, CCH, 2 * NPASS, 128], F8, isOutput=False)
    wv_d = nc.declare_dram_parameter("wv", [2, 128, CCH, C // 2], BF16, isOutput=False)
    bq64_d = nc.declare_dram_parameter("bq64", [128, CCH], F32, isOutput=False)
    bvb_d = nc.declare_dram_parameter("bvb", [128, C], BF16, isOutput=False)
    mask_d = nc.declare_dram_parameter("mask", [NB, 128, LCH], F32, isOutput=False)
    out_d = nc.declare_dram_parameter("out", [NB, LCH, 128, C], BF16, isOutput=True)

    with tile.TileContext(nc) as tc:
        with (
            tc.tile_pool(name="wpool", bufs=1) as wpool,
            tc.tile_pool(name="hpool", bufs=2) as hpool,
            tc.tile_pool(name="qkpool", bufs=2) as qkpool,
            tc.tile_pool(name="vpool", bufs=2 * LCH) as vpool,
            tc.tile_pool(name="epool", bufs=16) as epool,
            tc.tile_pool(name="cpool", bufs=2 * LCH) as cpool,
            tc.tile_pool(name="rpool", bufs=8) as rpool,
            tc.tile_pool(name="pqk", bufs=2, space=bass.MemorySpace.PSUM) as pqk,
            tc.tile_pool(name="pss", bufs=2, space=bass.MemorySpace.PSUM) as pss_pool,
            tc.tile_pool(name="ppv", bufs=2, space=bass.MemorySpace.PSUM) as ppv,
        ):
            # ---- persistent constants ----
            wq8 = wpool.tile([128, CCH, 2 * NPASS, 128], F8, tag="wq8")
            wk8 = wpool.tile([128, CCH, 2 * NPASS, 128], F8, tag="wk8")
            # wv split by output half so each half is one contiguous DMA
            # (a column-sliced DMA of one [128,CCH,C] tensor transfers in
            # 768B strided chunks and runs ~2x slower)
            wv = [wpool.tile([128, CCH, C // 2], BF16, tag=f"wv{h}", name=f"wv{h}")
                  for h in range(2)]
            bq64 = wpool.tile([128, CCH], F32, tag="bq64")
            bvb = wpool.tile([128, C], BF16, tag="bvb")
            hst8, hstb, msk, qt, kt, vt, e_of = {}, {}, {}, {}, {}, {}, {}
            for b in range(NB):
                hst8[b] = hpool.tile([128, 2 * NPASS, L], F8, tag="hst8", name=f"hst8_{b}")
                hstb[b] = hpool.tile([128, CCH, L], BF16, tag="hstb", name=f"hstb{b}")
                msk[b] = hpool.tile([128, LCH], F32, tag="mask", name=f"msk{b}")

            # staged input DMAs. Transfers on one queue run serially FIFO at
            # ~130GB/s; the three dynamic queues (sync/scalar/gpsimd) run
            # concurrently. Order per queue is a bandwidth schedule matched
            # to when each tensor gates PE work (first qk chunk ~9.5us,
            # v-projection pieces from ~15us, batch-1 tensors ~25us+).
            # sync: qk weights j0/j1 (contiguous j-pair slabs), rest of wq,
            # then wv halves
            nc.sync.dma_start(wq8[:, 0:2], wq8_d[:, 0:2])
            nc.sync.dma_start(wk8[:, 0:2], wk8_d[:, 0:2])
            nc.sync.dma_start(wq8[:, 2:6], wq8_d[:, 2:6])
            nc.sync.dma_start(wv[0][:], wv_d[0])
            nc.sync.dma_start(wv[1][:], wv_d[1])
            # scalar: hst8[0] as ONE contiguous transfer (pass-sliced DMAs
            # have 1KB strided runs and halve the queue's bandwidth), then
            # rest of wk, then b0 bf16 activations
            nc.scalar.dma_start(hst8[0][:], hst8_d[0])
            nc.scalar.dma_start(wk8[:, 2:6], wk8_d[:, 2:6])
            nc.scalar.dma_start(hstb[0][:, 0:3], hstb_d[0, :, 0:3])
            # gpsimd: small tensors, the other hstb[0] half, then batch 1
            nc.gpsimd.dma_start(msk[0][:], mask_d[0])
            nc.gpsimd.dma_start(bq64[:], bq64_d[:])
            nc.gpsimd.dma_start(hstb[0][:, 3:6], hstb_d[0, :, 3:6])
            nc.gpsimd.dma_start(bvb[:], bvb_d[:])
            nc.gpsimd.dma_start(msk[1][:], mask_d[1])
            nc.gpsimd.dma_start(hstb[1][:, 0:3], hstb_d[1, :, 0:3])
            nc.gpsimd.dma_start(hstb[1][:, 3:6], hstb_d[1, :, 3:6])
            nc.gpsimd.dma_start(hst8[1][:], hst8_d[1])

            def emit_v_piece(b, t, half):
                # one PSUM-group of the bf16 v projection: l-chunk t,
                # output half (6 of 12 heads). ~1us of PE filler.
                if half == 0 and t == 0:
                    vt[b] = [
                        vpool.tile([128, NH, DH + 1], BF16, tag="v", name=f"v{b}_{tt}")
                        for tt in range(LCH)
                    ]
                ncol = C // 2  # 384
                ps = pqk.tile([128, ncol], F32, tag="big", name="psv")
                for k in range(CCH):
                    nc.tensor.matmul(
                        ps[:],
                        hstb[b][:, k, 128 * t : 128 * (t + 1)],
                        wv[half][:, k, :],
                        start=(k == 0),
                        stop=(k == CCH - 1),
                    )
                nh2 = NH // 2
                nc.vector.tensor_add(
                    vt[b][t][:, half * nh2 : (half + 1) * nh2, 0:DH],
                    ps[:].rearrange("p (h d) -> p h d", d=DH),
                    bvb[:, half * ncol : (half + 1) * ncol].rearrange(
                        "p (h d) -> p h d", d=DH
                    ),
                )
                if half == 0:
                    nc.vector.memset(vt[b][t][:, :, DH : DH + 1], 1.0)

            def emit_qk_chunk(b, j):
                if j == 0:
                    qt[b] = qkpool.tile([128, CCH, L], BF16, tag="qt", name=f"qt{b}")
                    kt[b] = qkpool.tile([128, CCH, L], BF16, tag="kt", name=f"kt{b}")
                for w8, dst, bias in ((wq8, qt[b], bq64), (wk8, kt[b], None)):
                    ps = pqk.tile([128, L], F32, tag="big", name="psqk")
                    for p in range(NPASS):
                        nc.tensor.matmul(
                            ps[:],
                            w8[:, j, 2 * p : 2 * p + 2, :],
                            hst8[b][:, 2 * p : 2 * p + 2, :],
                            start=(p == 0),
                            stop=(p == NPASS - 1),
                            perf_mode=DR,
                        )
                    if bias is not None:
                        nc.vector.tensor_scalar_add(dst[:, j, :], ps[:], bias[:, j : j + 1])
                    else:
                        nc.vector.tensor_scalar_add(dst[:, j, :], ps[:], 0.0)

            def emit_scores_mc(b, hp, mc):
                if mc == 0:
                    e_of[(b, hp)] = [
                        epool.tile([128, 2, L], BF16, tag="e", name=f"e{b}_{hp}_{m}")
                        for m in range(LCH)
                    ]
                e = e_of[(b, hp)]
                ps = pss_pool.tile([128, 2, L], F32, tag="pss", name="pss")
                for h2 in range(2):
                    pr = slice(64 * h2, 64 * (h2 + 1))
                    nc.tensor.matmul(
                        ps[:, h2, :],
                        kt[b][pr, hp, 128 * mc : 128 * (mc + 1)],
                        qt[b][pr, hp, :],
                    )
                nc.scalar.activation(
                    e[mc][:].rearrange("p a l -> p (a l)"),
                    ps[:].rearrange("p a l -> p (a l)"), AF.Exp,
                    bias=msk[b][:, mc : mc + 1], scale=EXP_SCALE,
                )

            def emit_pv_pair(b, hp, lcs):
                # ctx-direct PV for a head pair over an l-chunk pair: one
                # PSUM bank holds both chunks' both heads' [128, 65]
                # results; denominators at column 64 of each -> one
                # [128,2,2] reciprocal + one broadcast-multiply into the
                # paired ctx tile.
                e = e_of[(b, hp)]
                pc = ppv.tile([128, 2, 2, DH + 1], F32, tag="pv", name="pc")
                for li, lc in enumerate(lcs):
                    for h2 in range(2):
                        head = 2 * hp + h2
                        for mc in range(LCH):
                            nc.tensor.matmul(
                                pc[:, li, h2, :],
                                e[mc][:, h2, 128 * lc : 128 * (lc + 1)],
                                vt[b][mc][:, head, :],
                                start=(mc == 0),
                                stop=(mc == LCH - 1),
                            )
                rec = rpool.tile([128, 2, 2, 1], F32, tag="rec", name="rec")
                nc.vector.reciprocal(rec[:], pc[:, :, :, DH : DH + 1])
                nc.vector.tensor_mul(
                    ctxs[(b, lcs[0] // 2)][:, :, 2 * hp : 2 * hp + 2, :],
                    pc[:, :, :, 0:DH],
                    rec[:].broadcast_to((128, 2, 2, DH)),
                )

            # ---- HAM warm-up: dummy matmuls on garbage SBUF keep the PE
            # active (half-clock) from program start so the activity monitor
            # releases the clock gate before the first real matmul ----
            warm = wpool.tile([128, 256], BF16, tag="warm")
            nc.vector.memset(warm[:], 0.0)
            # preload the Exp activation table off the critical path
            wexp = wpool.tile([128, 1], BF16, tag="wexp")
            nc.scalar.activation(wexp[:], warm[:, 0:1], AF.Exp, bias=0.0, scale=1.0)
            for i in range(16):
                pw = ppv.tile([1, 256], F32, tag="pv", name=f"pw{i}")
                nc.tensor.matmul(pw[:], warm[:, 0:1], warm[:])

            # ---- emission schedule ----
            units = []
            for b in range(NB):
                for hp in range(CCH):
                    units.append((b, hp))

            def emit_out(b, lcs):
                # spread the 0.2MB output DMAs across queues so no single
                # SWDGE issue chain (~0.7us each) sits on the critical path
                engs = {0: [nc.sync, nc.gpsimd, nc.sync, nc.gpsimd],
                        1: [nc.sync, nc.gpsimd, nc.scalar, nc.sync]}[b]
                for lc in lcs:
                    flat = ctxs[(b, lc // 2)][:, lc % 2].rearrange("p h d -> p (h d)")
                    engs[lc].dma_start(out_d[b, lc], flat[:])

            # ctx tiles paired over l-chunks (lc 0-1 and lc 2-3) so the PV
            # epilogue is one reciprocal + one multiply per head-pair
            ctxs = {}
            for b in range(NB):
                for lp in range(2):
                    ctxs[(b, lp)] = cpool.tile(
                        [128, 2, NH, DH], BF16, tag="ctx", name=f"ctx{b}_{lp}"
                    )
            from collections import deque
            # v-projection pieces double as PE filler between score groups:
            # unit 0 waits for hstb[0]/wv to land, so b0's pieces go in
            # units 0(end)-2, b1's spread across the middle units. qk for
            # unit N+1 is emitted inside unit N (between score groups) so
            # the ACT engine's exp stream never drains during a qk phase.
            # half-major order: PV for head-pair hp only reads v half hp//3,
            # so half-0 pieces unblock the first PV pairs while half-1
            # transfers are still in flight.
            vq = deque([(b, t, h) for b in range(NB) for h in range(2) for t in range(LCH)])
            # pieces popped after score-group 1 / group 2 of each unit;
            # units 0-1 get none (the input DMAs are still streaming at
            # ~0.4MB/us aggregate until ~18us — scheduling v earlier just
            # stalls the in-order PE queue on hstb/wv), then two pieces at
            # the top of each of units 2-9. PV runs at lag 3 so each
            # half's pieces land a unit before its first consumer.
            VFILL = {n: (2, 0) for n in range(2, 10)}

            lag = deque()
            emit_qk_chunk(*units[0])
            for n, (b, hp) in enumerate(units):
                v1, v2 = VFILL.get(n, (0, 0))
                emit_scores_mc(b, hp, 0)
                emit_scores_mc(b, hp, 1)
                for _ in range(v1):
                    if vq:
                        emit_v_piece(*vq.popleft())
                if len(lag) >= 3:
                    emit_pv_pair(*lag[0], (0, 1))
                emit_scores_mc(b, hp, 2)
                if n + 1 < len(units):
                    emit_qk_chunk(*units[n + 1])
                emit_scores_mc(b, hp, 3)
                for _ in range(v2):
                    if vq:
                        emit_v_piece(*vq.popleft())
                if len(lag) >= 3:
                    pp = lag.popleft()
                    emit_pv_pair(*pp, (2, 3))
                    e_of.pop(pp)
                    if pp[1] == CCH - 1:
                        emit_out(pp[0], (0, 1, 2, 3))
                lag.append((b, hp))
            di = 0
            while lag:
                pp = lag.popleft()
                emit_pv_pair(*pp, (0, 1))
                if pp[1] == CCH - 1:
                    emit_out(pp[0], (0, 1))
                # dummy matmuls between drain groups hold the clock gate
                # open through the tail (the PE would otherwise idle >3.4us
                # waiting on the last exps and drop to half clock)
                pw = ppv.tile([1, 256], F32, tag="pv", name=f"pwt{di}")
                nc.tensor.matmul(pw[:], warm[:, 0:1], warm[:])
                di += 1
                emit_pv_pair(*pp, (2, 3))
                e_of.pop(pp)
                if pp[1] == CCH - 1:
                    emit_out(pp[0], (2, 3))

    nc.compile()
    return nc


def _get_compiled():
    global _COMPILED
    if _COMPILED is None:
        _COMPILED = _build()
    return _COMPILED


def _prep_core(hs, mask, wq, wk, wv, bq, bv, g, b0):
    hs_gb = np.ascontiguousarray(hs[g, b0 : b0 + NB])  # [2, L, C]
    hs_cl = hs_gb.transpose(0, 2, 1)  # [2, C, L]
    # bf16 for the v projection: [b, p, k, l], c = 128k + p
    hstb = np.ascontiguousarray(
        hs_cl.reshape(NB, CCH, 128, L).transpose(0, 2, 1, 3)
    ).astype(NPBF16)
    # fp8 for q/k DoubleRow: [b, p, pass*2+i, l], c = 256*pass + 128*i + p
    hst8 = np.ascontiguousarray(
        hs_cl.reshape(NB, NPASS * 2, 128, L).transpose(0, 2, 1, 3)
    ).astype(NPF8)

    def wprep8(w):
        # [p, j, pass*2+i, d], c = 256*pass + 128*i + p, d = 128j + d'
        return np.ascontiguousarray(
            (w[g] * WSCALE).reshape(NPASS * 2, 128, CCH, 128).transpose(1, 2, 0, 3)
        ).astype(NPF8)

    def wprep(w):
        # output-half-major: [h, p, k, d'] = W[128k+p, 384h+d']
        return np.ascontiguousarray(
            w[g].reshape(CCH, 128, 2, C // 2).transpose(2, 1, 0, 3)
        ).astype(NPBF16)

    bq_t = np.ascontiguousarray(bq[g, 0].reshape(CCH, 128).T * WSCALE).astype(np.float32)
    bvb = np.ascontiguousarray(np.broadcast_to(bv[g, 0], (128, C))).astype(NPBF16)
    # mask[b, p, mc] = mask[g, b0+b, 0, 0, 128mc+p]
    msk = np.ascontiguousarray(
        mask[g, b0 : b0 + NB, 0, 0].reshape(NB, LCH, 128).transpose(0, 2, 1)
    ).astype(np.float32)
    return {
        "hst8": hst8,
        "hstb": hstb,
        "wq8": wprep8(wq),
        "wk8": wprep8(wk),
        "wv": wprep(wv),
        "bq64": bq_t,
        "bvb": bvb,
        "mask": msk,
    }


def kernel(
    hidden_states,
    attention_mask,
    query_weight,
    query_bias,
    key_weight,
    key_bias,
    value_weight,
    value_bias,
    _trace=False,
):
    hs = np.asarray(hidden_states, dtype=np.float32)
    mask = np.asarray(attention_mask, dtype=np.float32)
    wq = np.asarray(query_weight, dtype=np.float32)
    wk = np.asarray(key_weight, dtype=np.float32)
    wv = np.asarray(value_weight, dtype=np.float32)
    bq = np.asarray(query_bias, dtype=np.float32)
    bv = np.asarray(value_bias, dtype=np.float32)
    del key_bias  # exactly cancelled by the softmax normalization

    nc = _get_compiled()
    in_maps = []
    for c in range(N_CORES):
        g, b0 = c // 2, NB * (c % 2)
        in_maps.append(_prep_core(hs, mask, wq, wk, wv, bq, bv, g, b0))

    global _COMPILED
    res = None
    for attempt in range(3):
        try:
            res = bass_utils.run_bass_kernel_spmd(
                nc, in_maps, core_ids=list(range(N_CORES)), trace=_trace
            )
            # force materialization so device faults surface here
            for m in res.results:
                for v in m.values():
                    np.asarray(v)
            break
        except Exception:
            if attempt == 2:
                raise
            _COMPILED = None
            nc = _get_compiled()

    out = np.empty((G, B, L, C), dtype=np.float32)
    for c in range(N_CORES):
        g, b0 = c // 2, NB * (c % 2)
        o = res.results[c]["out"]  # [NB, LCH, 128, C] bf16
        out[g, b0 : b0 + NB] = o.reshape(NB, L, C).astype(np.float32)
    if _trace:
        kernel.last_exec_time_ns = res.exec_time_ns
    return out


# revision 37
# speedup vs baseline: 1.0372x; 1.0194x over previous
"""Grouped BERT self-attention on 8 TRN2 NeuronCores.

Problem: G=4 groups, B=4 batch, L=512 seq, C=768 (12 heads x 64).
Sharding: the 16 (g, b) attention problems are embarrassingly parallel;
each core handles one group g = core//2 and two batches. Weights are
per-group so each core loads exactly one group's weights. No collectives.

Per-(g,b) on-chip dataflow:
  q/k projections run in fp8(e4m3) DoubleRow: weights are host-scaled by
  64 (fp8 dynamic range) and packed [p, j, pass*2+i, d] so each DR matmul
  contracts 256 of C=768 per pass (3 passes/chunk, 2x bf16 throughput).
  The k bias is dropped entirely: the resulting per-query multiplicative
  factor exp(q.bk) cancels in the softmax normalization. The q bias is
  pre-scaled by 64 host-side so the PSUM drain stays a single
  tensor_scalar_add; the 64*64 scaling is folded into the exp scale
  (0.125/4096).
    qT[d,l] = (64 Wq[c,d]).T @ hst8[c,l] + 64 bq   (fp8 DR, bf16 out)
    kT[d,l] = (64 Wk[c,d]).T @ hst8[c,l]           (fp8 DR, bf16 out)
  v[m,d] = hstb[c,m].T @ Wv[c,d] + bv  stays bf16 (v feeds the output
    average directly; fp8 there would blow the 2e-2 error budget), stored
    [m, head, 65] with a ones column per head -> softmax denominator.
  ST[m,l] = kT[d,m].T @ qT[d,l]  (heads paired on partitions 0:64/64:128
    -> concurrent PE row-tiles, shared 2-bank PSUM tile)
  E[m,l]  = exp(ST/32768 + mask[m])  (one ACT op per head-pair, bf16 out)
  ctx[l, 2, d+1] = E[m,l].T @ v_aug[m, d+1]  (ctx-direct; column d=64
    catches the softmax denominator)
  out[l,:] = ctx * recip(denom)

PE emission interleaves score-pair matmuls of unit N with the PV matmuls
of unit N-2 so the in-order PE queue never waits on the ScalarEngine's
exp, and spreads the bf16 v-projection matmuls across units as PE filler
(the fp8 q/k path alone would under-fill the PE against the exp cadence
and let the HAM clock gate re-throttle mid-kernel). Input DMAs are
split/staged so tensors gating the first matmul group transfer first,
and dummy warm-up matmuls hold the PE activity monitor at full clock
until real work arrives.
"""

import numpy as np
import ml_dtypes

import concourse.bacc as bacc
import concourse.bass as bass
import concourse.tile as tile
import concourse.mybir as mybir
from concourse import bass_utils

# avoid FishPath artifact upload in the axon trace path
bass_utils.upload_artifacts = lambda tmpdir: tmpdir

G, B, L, C = 4, 4, 512, 768
NH, DH = 12, 64
NB = 2          # batches per core
CCH = C // 128  # 6 contraction chunks
NPASS = 3       # fp8 DoubleRow passes (256 contraction each)
LCH = L // 128  # 4 seq chunks
N_CORES = 8
WSCALE = 64.0   # fp8 weight pre-scale
EXP_SCALE = 0.125 / (WSCALE * WSCALE)

BF16 = mybir.dt.bfloat16
F8 = mybir.dt.float8e4
F32 = mybir.dt.float32
NPBF16 = ml_dtypes.bfloat16
NPF8 = ml_dtypes.float8_e4m3

_COMPILED = None


def _build():
    nc = bacc.Bacc("TRN2", target_bir_lowering=False, debug=False)
    AF = mybir.ActivationFunctionType
    DR = mybir.MatmulPerfMode.DoubleRow

    hst8_d = nc.declare_dram_parameter("hst8", [NB, 128, 2 * NPASS, L], F8, isOutput=False)
    hstb_d = nc.declare_dram_parameter("hstb", [NB, 128, CCH, L], BF16, isOutput=False)
    wq8_d = nc.declare_dram_parameter("wq8", [128, CCH, 2 * NPASS, 128], F8, isOutput=False)
    wk8_d = nc.declare_dram_parameter("wk8", [128, CCH, 2 * NPASS, 128], F8, isOutput=False)
    wv_d = nc.declare_dram_parameter("wv", [2, 128, CCH, C // 2], BF16, isOutput=False)
    bq64_d = nc.declare_dram_parameter("bq64", [128, CCH], F32, isOutput=False)
    bvb_d = nc.declare_dram_parameter("bvb", [128, C], BF16, isOutput=False)
    mask_d = nc.declare_dram_parameter("mask", [NB, 128, LCH], F32, isOutput=False)
    out_d = nc.declare_dram_parameter("out", [NB, LCH, 128, C], BF16, isOutput=True)

    with tile.TileContext(nc) as tc:
        with (
            tc.tile_pool(name="wpool", bufs=1) as wpool,
            tc.tile_pool(name="hpool", bufs=2) as hpool,
            tc.tile_pool(name="qkpool", bufs=2) as qkpool,
            tc.tile_pool(name="vpool", bufs=2 * LCH) as vpool,
            tc.tile_pool(name="epool", bufs=20) as epool,
            tc.tile_pool(name="cpool", bufs=2 * LCH) as cpool,
            tc.tile_pool(name="rpool", bufs=8) as rpool,
            tc.tile_pool(name="pqk", bufs=2, space=bass.MemorySpace.PSUM) as pqk,
            tc.tile_pool(name="pss", bufs=2, space=bass.MemorySpace.PSUM) as pss_pool,
            tc.tile_pool(name="ppv", bufs=2, space=bass.MemorySpace.PSUM) as ppv,
        ):
            # ---- persistent constants ----
            wq8 = wpool.tile([128, CCH, 2 * NPASS, 128], F8, tag="wq8")
            wk8 = wpool.tile([128, CCH, 2 * NPASS, 128], F8, tag="wk8")
            # wv split by output half so each half is one contiguous DMA
            # (a column-sliced DMA of one [128,CCH,C] tensor transfers in
            # 768B strided chunks and runs ~2x slower)
            wv = [wpool.tile([128, CCH, C // 2], BF16, tag=f"wv{h}", name=f"wv{h}")
                  for h in range(2)]
            bq64 = wpool.tile([128, CCH], F32, tag="bq64")
            bvb = wpool.tile([128, C], BF16, tag="bvb")
            hst8, hstb, msk, qt, kt, vt, e_of = {}, {}, {}, {}, {}, {}, {}
            for b in range(NB):
                hst8[b] = hpool.tile([128, 2 * NPASS, L], F8, tag="hst8", name=f"hst8_{b}")
                hstb[b] = hpool.tile([128, CCH, L], BF16, tag="hstb", name=f"hstb{b}")
                msk[b] = hpool.tile([128, LCH], F32, tag="mask", name=f"msk{b}")

            # staged input DMAs. Transfers on one queue run serially FIFO at
            # ~130GB/s; the three dynamic queues (sync/scalar/gpsimd) run
            # concurrently. Order per queue is a bandwidth schedule matched
            # to when each tensor gates PE work (first qk chunk ~9.5us,
            # v-projection pieces from ~15us, batch-1 tensors ~25us+).
            # sync: qk weights j0/j1 (contiguous j-pair slabs), rest of wq,
            # then wv halves
            nc.sync.dma_start(wq8[:, 0:2], wq8_d[:, 0:2])
            nc.sync.dma_start(wk8[:, 0:2], wk8_d[:, 0:2])
            nc.sync.dma_start(wq8[:, 2:6], wq8_d[:, 2:6])
            nc.sync.dma_start(wv[0][:], wv_d[0])
            nc.sync.dma_start(wv[1][:], wv_d[1])
            # scalar: hst8[0] as ONE contiguous transfer (pass-sliced DMAs
            # have 1KB strided runs and halve the queue's bandwidth), then
            # rest of wk, then b0 bf16 activations
            nc.scalar.dma_start(hst8[0][:], hst8_d[0])
            nc.scalar.dma_start(wk8[:, 2:6], wk8_d[:, 2:6])
            nc.scalar.dma_start(hstb[0][:, 0:3], hstb_d[0, :, 0:3])
            # gpsimd: small tensors, the other hstb[0] half, then batch 1
            nc.gpsimd.dma_start(msk[0][:], mask_d[0])
            nc.gpsimd.dma_start(bq64[:], bq64_d[:])
            nc.gpsimd.dma_start(bvb[:], bvb_d[:])
            nc.gpsimd.dma_start(hstb[0][:, 3:6], hstb_d[0, :, 3:6])
            nc.gpsimd.dma_start(msk[1][:], mask_d[1])
            nc.gpsimd.dma_start(hstb[1][:, 0:3], hstb_d[1, :, 0:3])
            nc.gpsimd.dma_start(hstb[1][:, 3:6], hstb_d[1, :, 3:6])
            nc.gpsimd.dma_start(hst8[1][:], hst8_d[1])

            def emit_v_piece(b, t, half):
                # one PSUM-group of the bf16 v projection: l-chunk t,
                # output half (6 of 12 heads). ~1us of PE filler.
                if half == 0 and t == 0:
                    vt[b] = [
                        vpool.tile([128, NH, DH + 1], BF16, tag="v", name=f"v{b}_{tt}")
                        for tt in range(LCH)
                    ]
                ncol = C // 2  # 384
                ps = pqk.tile([128, ncol], F32, tag="big", name="psv")
                for k in range(CCH):
                    nc.tensor.matmul(
                        ps[:],
                        hstb[b][:, k, 128 * t : 128 * (t + 1)],
                        wv[half][:, k, :],
                        start=(k == 0),
                        stop=(k == CCH - 1),
                    )
                nh2 = NH // 2
                nc.vector.tensor_add(
                    vt[b][t][:, half * nh2 : (half + 1) * nh2, 0:DH],
                    ps[:].rearrange("p (h d) -> p h d", d=DH),
                    bvb[:, half * ncol : (half + 1) * ncol].rearrange(
                        "p (h d) -> p h d", d=DH
                    ),
                )
                if half == 0:
                    nc.vector.memset(vt[b][t][:, :, DH : DH + 1], 1.0)

            def emit_qk_chunk(b, j):
                if j == 0:
                    qt[b] = qkpool.tile([128, CCH, L], BF16, tag="qt", name=f"qt{b}")
                    kt[b] = qkpool.tile([128, CCH, L], BF16, tag="kt", name=f"kt{b}")
                for w8, dst, bias in ((wq8, qt[b], bq64), (wk8, kt[b], None)):
                    ps = pqk.tile([128, L], F32, tag="big", name="psqk")
                    for p in range(NPASS):
                        nc.tensor.matmul(
                            ps[:],
                            w8[:, j, 2 * p : 2 * p + 2, :],
                            hst8[b][:, 2 * p : 2 * p + 2, :],
                            start=(p == 0),
                            stop=(p == NPASS - 1),
                            perf_mode=DR,
                        )
                    if bias is not None:
                        nc.vector.tensor_scalar_add(dst[:, j, :], ps[:], bias[:, j : j + 1])
                    else:
                        nc.vector.tensor_scalar_add(dst[:, j, :], ps[:], 0.0)

            def emit_scores_mc(b, hp, mc):
                if mc == 0:
                    e_of[(b, hp)] = [
                        epool.tile([128, 2, L], BF16, tag="e", name=f"e{b}_{hp}_{m}")
                        for m in range(LCH)
                    ]
                e = e_of[(b, hp)]
                ps = pss_pool.tile([128, 2, L], F32, tag="pss", name="pss")
                for h2 in range(2):
                    pr = slice(64 * h2, 64 * (h2 + 1))
                    nc.tensor.matmul(
                        ps[:, h2, :],
                        kt[b][pr, hp, 128 * mc : 128 * (mc + 1)],
                        qt[b][pr, hp, :],
                    )
                nc.scalar.activation(
                    e[mc][:].rearrange("p a l -> p (a l)"),
                    ps[:].rearrange("p a l -> p (a l)"), AF.Exp,
                    bias=msk[b][:, mc : mc + 1], scale=EXP_SCALE,
                )

            def emit_pv_pair(b, hp, lcs):
                # ctx-direct PV for a head pair over an l-chunk pair: one
                # PSUM bank holds both chunks' both heads' [128, 65]
                # results; denominators at column 64 of each -> one
                # [128,2,2] reciprocal + one broadcast-multiply into the
                # paired ctx tile.
                e = e_of[(b, hp)]
                pc = ppv.tile([128, 2, 2, DH + 1], F32, tag="pv", name="pc")
                for li, lc in enumerate(lcs):
                    for h2 in range(2):
                        head = 2 * hp + h2
                        for mc in range(LCH):
                            nc.tensor.matmul(
                                pc[:, li, h2, :],
                                e[mc][:, h2, 128 * lc : 128 * (lc + 1)],
                                vt[b][mc][:, head, :],
                                start=(mc == 0),
                                stop=(mc == LCH - 1),
                            )
                rec = rpool.tile([128, 2, 2, 1], F32, tag="rec", name="rec")
                nc.vector.reciprocal(rec[:], pc[:, :, :, DH : DH + 1])
                nc.vector.tensor_mul(
                    ctxs[(b, lcs[0] // 2)][:, :, 2 * hp : 2 * hp + 2, :],
                    pc[:, :, :, 0:DH],
                    rec[:].broadcast_to((128, 2, 2, DH)),
                )

            # ---- HAM warm-up: dummy matmuls on garbage SBUF keep the PE
            # active (half-clock) from program start so the activity monitor
            # releases the clock gate before the first real matmul ----
            warm = wpool.tile([128, 256], BF16, tag="warm")
            nc.vector.memset(warm[:], 0.0)
            # preload the Exp activation table off the critical path
            wexp = wpool.tile([128, 1], BF16, tag="wexp")
            nc.scalar.activation(wexp[:], warm[:, 0:1], AF.Exp, bias=0.0, scale=1.0)
            for i in range(16):
                pw = ppv.tile([1, 256], F32, tag="pv", name=f"pw{i}")
                nc.tensor.matmul(pw[:], warm[:, 0:1], warm[:])

            # ---- emission schedule ----
            units = []
            for b in range(NB):
                for hp in range(CCH):
                    units.append((b, hp))

            def emit_out(b, lcs):
                # spread the 0.2MB output DMAs across queues so no single
                # SWDGE issue chain (~0.7us each) sits on the critical path
                engs = {0: [nc.sync, nc.gpsimd, nc.sync, nc.gpsimd],
                        1: [nc.sync, nc.gpsimd, nc.scalar, nc.sync]}[b]
                for lc in lcs:
                    flat = ctxs[(b, lc // 2)][:, lc % 2].rearrange("p h d -> p (h d)")
                    engs[lc].dma_start(out_d[b, lc], flat[:])

            # ctx tiles paired over l-chunks (lc 0-1 and lc 2-3) so the PV
            # epilogue is one reciprocal + one multiply per head-pair
            ctxs = {}
            for b in range(NB):
                for lp in range(2):
                    ctxs[(b, lp)] = cpool.tile(
                        [128, 2, NH, DH], BF16, tag="ctx", name=f"ctx{b}_{lp}"
                    )
            from collections import deque
            # v-projection pieces double as PE filler between score groups:
            # unit 0 waits for hstb[0]/wv to land, so b0's pieces go in
            # units 0(end)-2, b1's spread across the middle units. qk for
            # unit N+1 is emitted inside unit N (between score groups) so
            # the ACT engine's exp stream never drains during a qk phase.
            # half-major order: PV for head-pair hp only reads v half hp//3,
            # so half-0 pieces unblock the first PV pairs while half-1
            # transfers are still in flight.
            vq = deque([(b, t, h) for b in range(NB) for h in range(2) for t in range(LCH)])
            # pieces popped after score-group 1 / group 2 of each unit;
            # units 0-1 get none (the input DMAs are still streaming at
            # ~0.4MB/us aggregate until ~18us — scheduling v earlier just
            # stalls the in-order PE queue on hstb/wv), then one piece per
            # score-group in units 2-9 (two pieces back-to-back before mc2
            # out-paces the exp double-buffer and starves the ACT engine).
            # PV runs at lag 4 so each half's pieces land a unit before
            # its first consumer.
            VFILL = {n: (1, 1) for n in range(2, 10)}

            lag = deque()
            emit_qk_chunk(*units[0])
            for n, (b, hp) in enumerate(units):
                v1, v2 = VFILL.get(n, (0, 0))
                emit_scores_mc(b, hp, 0)
                emit_scores_mc(b, hp, 1)
                for _ in range(v1):
                    if vq:
                        emit_v_piece(*vq.popleft())
                if len(lag) >= 4:
                    emit_pv_pair(*lag[0], (0, 1))
                emit_scores_mc(b, hp, 2)
                if n + 1 < len(units):
                    emit_qk_chunk(*units[n + 1])
                emit_scores_mc(b, hp, 3)
                for _ in range(v2):
                    if vq:
                        emit_v_piece(*vq.popleft())
                if len(lag) >= 4:
                    pp = lag.popleft()
                    emit_pv_pair(*pp, (2, 3))
                    e_of.pop(pp)
                    if pp[1] == CCH - 1:
                        emit_out(pp[0], (0, 1, 2, 3))
                lag.append((b, hp))
            # drain: run every pending unit's (0,1) PV first, then ship the
            # l-chunk-0/1 outputs while the (2,3) PV work (plus dummy
            # clock-gate matmuls) still occupies the engines.
            drain = list(lag)
            for di, pp in enumerate(drain):
                emit_pv_pair(*pp, (0, 1))
                pw = ppv.tile([1, 256], F32, tag="pv", name=f"pwt{di}")
                nc.tensor.matmul(pw[:], warm[:, 0:1], warm[:])
            emit_out(drain[-1][0], (0, 1))
            for pp in drain:
                emit_pv_pair(*pp, (2, 3))
                e_of.pop(pp)
            emit_out(drain[-1][0], (2, 3))

    nc.compile()
    return nc


def _get_compiled():
    global _COMPILED
    if _COMPILED is None:
        _COMPILED = _build()
    return _COMPILED


def _prep_core(hs, mask, wq, wk, wv, bq, bv, g, b0):
    hs_gb = np.ascontiguousarray(hs[g, b0 : b0 + NB])  # [2, L, C]
    hs_cl = hs_gb.transpose(0, 2, 1)  # [2, C, L]
    # bf16 for the v projection: [b, p, k, l], c = 128k + p
    hstb = np.ascontiguousarray(
        hs_cl.reshape(NB, CCH, 128, L).transpose(0, 2, 1, 3)
    ).astype(NPBF16)
    # fp8 for q/k DoubleRow: [b, p, pass*2+i, l], c = 256*pass + 128*i + p
    hst8 = np.ascontiguousarray(
        hs_cl.reshape(NB, NPASS * 2, 128, L).transpose(0, 2, 1, 3)
    ).astype(NPF8)

    def wprep8(w):
        # [p, j, pass*2+i, d], c = 256*pass + 128*i + p, d = 128j + d'
        return np.ascontiguousarray(
            (w[g] * WSCALE).reshape(NPASS * 2, 128, CCH, 128).transpose(1, 2, 0, 3)
        ).astype(NPF8)

    def wprep(w):
        # output-half-major: [h, p, k, d'] = W[128k+p, 384h+d']
        return np.ascontiguousarray(
            w[g].reshape(CCH, 128, 2, C // 2).transpose(2, 1, 0, 3)
        ).astype(NPBF16)

    bq_t = np.ascontiguousarray(bq[g, 0].reshape(CCH, 128).T * WSCALE).astype(np.float32)
    bvb = np.ascontiguousarray(np.broadcast_to(bv[g, 0], (128, C))).astype(NPBF16)
    # mask[b, p, mc] = mask[g, b0+b, 0, 0, 128mc+p]
    msk = np.ascontiguousarray(
        mask[g, b0 : b0 + NB, 0, 0].reshape(NB, LCH, 128).transpose(0, 2, 1)
    ).astype(np.float32)
    return {
        "hst8": hst8,
        "hstb": hstb,
        "wq8": wprep8(wq),
        "wk8": wprep8(wk),
        "wv": wprep(wv),
        "bq64": bq_t,
        "bvb": bvb,
        "mask": msk,
    }


def kernel(
    hidden_states,
    attention_mask,
    query_weight,
    query_bias,
    key_weight,
    key_bias,
    value_weight,
    value_bias,
    _trace=False,
):
    hs = np.asarray(hidden_states, dtype=np.float32)
    mask = np.asarray(attention_mask, dtype=np.float32)
    wq = np.asarray(query_weight, dtype=np.float32)
    wk = np.asarray(key_weight, dtype=np.float32)
    wv = np.asarray(value_weight, dtype=np.float32)
    bq = np.asarray(query_bias, dtype=np.float32)
    bv = np.asarray(value_bias, dtype=np.float32)
    del key_bias  # exactly cancelled by the softmax normalization

    nc = _get_compiled()
    in_maps = []
    for c in range(N_CORES):
        g, b0 = c // 2, NB * (c % 2)
        in_maps.append(_prep_core(hs, mask, wq, wk, wv, bq, bv, g, b0))

    global _COMPILED
    res = None
    for attempt in range(3):
        try:
            res = bass_utils.run_bass_kernel_spmd(
                nc, in_maps, core_ids=list(range(N_CORES)), trace=_trace
            )
            # force materialization so device faults surface here
            for m in res.results:
                for v in m.values():
                    np.asarray(v)
            break
        except Exception:
            if attempt == 2:
                raise
            _COMPILED = None
            nc = _get_compiled()

    out = np.empty((G, B, L, C), dtype=np.float32)
    for c in range(N_CORES):
        g, b0 = c // 2, NB * (c % 2)
        o = res.results[c]["out"]  # [NB, LCH, 128, C] bf16
        out[g, b0 : b0 + NB] = o.reshape(NB, L, C).astype(np.float32)
    if _trace:
        kernel.last_exec_time_ns = res.exec_time_ns
    return out
